# revision 15
# baseline (speedup 1.0000x reference)
"""Distributed causal multi-head attention block for 8 TRN2 NeuronCores, v3.

Problem: y = proj(softmax_causal((x Wq)(x Wk)^T / 8) (x Wv)) with
B=1, S=4096, D=1024, H=16 heads, Dh=64, all float32.

Sharding (head-parallel attention + query-row-split projection):
- Each core c owns heads {2c, 2c+1}; x replicated (transposed on host).
- Attention output re-sharded head-major -> query-major with FOUR chunk-pair
  AllToAlls; core c ends up owning rows [64c, 64c+64) of every 512-q chunk.
- Each core projects its 64-row blocks through full Wproj; host interleaves.

Schedule (trace-driven):
- The Activation engine (exp, ~150us busy) is the serial floor. The kt loop
  is software-pipelined with one-iteration lag: while exp(kt) runs, the PE
  does sc(kt+1) and av(kt-1) plus filler; sc(chunk+1, 0) is emitted in the
  last iteration of the previous chunk so exps flow across boundaries.
- QKV projections of chunks 0-2 are prestaged before attention(0); the rest
  flow through a global filler queue with per-iteration budget and split
  qk/v deadline tags (early chunks' exp windows cannot hide a full QKV
  projection; oversized budgets stall the scalar engine instead).
- Diagonal exps use a strided [128, 2, n] AP over the two head segments so
  the dead gap columns are not exp'd.
- A2A groups are 4 chunk pairs. proj(2)'s a2a_out load happens at chunk 6;
  its matmuls run during the last A2A's flight (at-load for proj(3) goes on
  the gpsimd queue so the sync queue never head-of-line blocks the tail).
- Chunk 7 normalizes/ships in four 128-column waves as each diagonal tile
  completes, reading av PSUM directly and broadcasting the reciprocal rows
  via tiny PE outer products (gpsimd broadcasts are ~0.7us each and would
  serialize in the tail); each wave ships head0 on sync, head1 on gpsimd.
"""

import sys

sys.path.insert(0, "/opt/trn_rl_repo")

import numpy as np
import ml_dtypes

from concourse import bacc, tile, mybir
from concourse import bass_utils
from concourse.bass_utils import run_bass_kernel_spmd

bass_utils.upload_artifacts = lambda tmpdir: tmpdir  # no S3 in this container

dt = mybir.dt
AF = mybir.ActivationFunctionType

N_CORES = 8
S = 4096
D = 1024
P = 128
CH = 512            # seq chunk (query block per iteration)
NCHUNK = S // CH    # 8
KT_PER_CH = CH // P  # 4
GROUPS = [(0, 1), (2, 3), (4, 5), (6, 7)]
CHUNK_GROUP = {c: (g, gi.index(c)) for g, gi in enumerate(GROUPS) for c in gi}

_BUILD_CACHE = {}


def _build(has_bq: bool, has_bp: bool):
    key = (has_bq, has_bp)
    if key in _BUILD_CACHE:
        return _BUILD_CACHE[key]

    nc = bacc.Bacc("TRN2", target_bir_lowering=False, debug=False,
                   num_devices=N_CORES)

    f32, bf16 = dt.float32, dt.bfloat16

    # ---- external I/O (per-core values supplied via in_maps) ----
    xT_ext = nc.dram_tensor("xT", [NCHUNK, NCHUNK, P, CH], bf16, kind="ExternalInput")
    wq_ext = nc.dram_tensor("wq", [NCHUNK, P, P], bf16, kind="ExternalInput")
    wk_ext = nc.dram_tensor("wk", [NCHUNK, P, P], bf16, kind="ExternalInput")
    wv_ext = nc.dram_tensor("wv", [NCHUNK, P, P], bf16, kind="ExternalInput")
    wp_ext = nc.dram_tensor("wp", [NCHUNK, P, D], bf16, kind="ExternalInput")
    bq_ext = nc.dram_tensor("bq", [P, 3], f32, kind="ExternalInput")
    bp_ext = nc.dram_tensor("bp", [1, D], f32, kind="ExternalInput")
    bv_ext = nc.dram_tensor("bv", [1, P], bf16, kind="ExternalInput")
    out_ext = nc.dram_tensor("out", [NCHUNK, 64, D], f32, kind="ExternalOutput")

    # one shared 128x128 diagonal triangle mask: mask[k, q] = 1 if k <= q.
    kk = np.arange(P)[:, None]
    qq = np.arange(P)[None, :]
    masks_np = (kk <= qq).astype(ml_dtypes.bfloat16)   # [128, 128]
    masks_dram = nc.inline_tensor(masks_np, name="masks_const")
    ones_np = np.ones((P, P), dtype=np.float32)
    ones_dram = nc.inline_tensor(ones_np, name="ones_const")

    with tile.TileContext(nc) as tc:
        with tc.tile_pool(name="const", bufs=1) as const, \
             tc.tile_pool(name="wpool", bufs=1) as wpool, \
             tc.tile_pool(name="resid", bufs=1) as resid, \
             tc.tile_pool(name="xp", bufs=2) as xp, \
             tc.tile_pool(name="probs", bufs=6) as probsp, \
             tc.tile_pool(name="small", bufs=2) as smallp, \
             tc.tile_pool(name="attnp", bufs=4) as attnp, \
             tc.tile_pool(name="atp", bufs=2) as atp, \
             tc.tile_pool(name="outp", bufs=4) as outpool, \
             tc.tile_pool(name="psA", bufs=2, space="PSUM") as psA, \
             tc.tile_pool(name="psS", bufs=2, space="PSUM") as psS, \
             tc.tile_pool(name="psV", bufs=2, space="PSUM") as psV, \
             tc.tile_pool(name="dram", bufs=1, space="DRAM") as dram:

            # ---- constants ----
            ones_bf_sb = const.tile([1, P], bf16)
            nc.vector.memset(ones_bf_sb[:], 1.0)
            if has_bp:
                ones_r_sb = const.tile([1, P], f32)
                nc.sync.dma_start(ones_r_sb[:], ones_dram.ap()[0:1, :])
                bp_sb = const.tile([1, D], f32)
                nc.sync.dma_start(bp_sb[:], bp_ext.ap())
            if has_bq:
                bq_sb = const.tile([P, 3], f32)
                nc.sync.dma_start(bq_sb[:], bq_ext.ap())
                bv_sb = const.tile([1, P], bf16)
                nc.sync.dma_start(bv_sb[:], bv_ext.ap())

            wq_sb = wpool.tile([P, NCHUNK, P], bf16)
            wk_sb = wpool.tile([P, NCHUNK, P], bf16)
            wv_sb = wpool.tile([P, NCHUNK, P], bf16)
            wp_sb = wpool.tile([P, NCHUNK, D], bf16)
            masks_sb = const.tile([P, P], bf16)

            qkt_tiles = []  # [128, 1024]: cols 0:512 Q^T, 512:1024 K^T
            v_tiles = []    # [128, 4, 130]: per ktile cols 0:64 head-a V,
                            # 64 ones, 65:129 head-b V, 129 ones
            for c in range(NCHUNK):
                qkt_tiles.append(resid.tile([P, 2 * CH], bf16, name=f"qkt{c}"))
                v_tiles.append(resid.tile([P, KT_PER_CH, 130], bf16,
                                          name=f"v{c}"))

            a2a_in = [dram.tile([N_CORES, P, P], bf16, name=f"a2a_in{g}")
                      for g in range(len(GROUPS))]
            a2a_out = [dram.tile([N_CORES, P, P], bf16, name=f"a2a_out{g}")
                       for g in range(len(GROUPS))]

            x_tiles_all = {}

            def emit_x_load(c, quarters=False):
                xt = xp.tile([P, NCHUNK, CH], bf16, tag="x", name=f"x{c}")
                step = 2 if quarters else 4
                for t0 in range(0, NCHUNK, step):
                    nc.sync.dma_start(
                        xt[:, t0:t0 + step, :],
                        xT_ext.ap()[c, t0:t0 + step].rearrange("t p q -> p t q"))
                x_tiles_all[c] = xt

            # ================= QKV projection items ========================
            def aq_items(c):
                state = {}

                def mk_mm(t0, t1):
                    def run():
                        if t0 == 0:
                            state["ps"] = psA.tile([P, CH], f32, tag="qkv",
                                                   name=f"psq{c}")
                        ps = state["ps"]
                        xt = x_tiles_all[c]
                        for t in (t0, t1):
                            nc.tensor.matmul(ps[:], wq_sb[:, t, :],
                                             xt[:, t, :],
                                             start=(t == 0),
                                             stop=(t == NCHUNK - 1))
                    return run

                def evict():
                    ps = state.pop("ps")
                    if has_bq:
                        nc.scalar.activation(qkt_tiles[c][:, 0:CH], ps[:],
                                             AF.Copy,
                                             bias=bq_sb[:, 0][:, None])
                    else:
                        nc.vector.tensor_copy(qkt_tiles[c][:, 0:CH], ps[:])
                return [(1024, mk_mm(0, 1)), (1024, mk_mm(2, 3)),
                        (1024, mk_mm(4, 5)), (1024, mk_mm(6, 7)),
                        (0, evict)]

            def k_items(c):
                state = {}

                def mk_kmm(t0, t1):
                    def run():
                        if t0 == 0:
                            state["ps"] = psA.tile([P, CH], f32, tag="qkv",
                                                   name=f"psk{c}")
                        ps = state["ps"]
                        xt = x_tiles_all[c]
                        for t in (t0, t1):
                            nc.tensor.matmul(ps[:], wk_sb[:, t, :],
                                             xt[:, t, :],
                                             start=(t == 0),
                                             stop=(t == NCHUNK - 1))
                    return run

                def kevict():
                    ps = state.pop("ps")
                    if has_bq:
                        nc.scalar.activation(qkt_tiles[c][:, CH:2 * CH], ps[:],
                                             AF.Copy,
                                             bias=bq_sb[:, 1][:, None])
                    else:
                        nc.vector.tensor_copy(qkt_tiles[c][:, CH:2 * CH],
                                              ps[:])
                return [(1024, mk_kmm(0, 1)), (1024, mk_kmm(2, 3)),
                        (1024, mk_kmm(4, 5)), (1024, mk_kmm(6, 7)),
                        (0, kevict)]

            def v_items(c):
                def mk_v(b):
                    def run():
                        psv = psA.tile([P, P], f32, tag="qkv",
                                       name=f"psv{c}_{b}")
                        xt = x_tiles_all[c]
                        if has_bq:
                            nc.tensor.matmul(psv[:], ones_bf_sb[0:1, :],
                                             bv_sb[0:1, :], start=True,
                                             stop=False)
                        for t in range(NCHUNK):
                            nc.tensor.matmul(
                                psv[:], xt[:, t, P * b:P * (b + 1)],
                                wv_sb[:, t, :],
                                start=(t == 0 and not has_bq),
                                stop=(t == NCHUNK - 1))
                        nc.vector.tensor_copy(v_tiles[c][:, b, 0:64],
                                              psv[:, 0:64])
                        nc.vector.tensor_copy(v_tiles[c][:, b, 65:129],
                                              psv[:, 64:128])
                    return run

                def vdone():
                    nc.vector.memset(v_tiles[c][:, :, 64:65], 1.0)
                    nc.vector.memset(v_tiles[c][:, :, 129:130], 1.0)
                    x_tiles_all.pop(c)

                return [(1024, mk_v(b)) for b in range(4)] + [(0, vdone)]

            # ================= projection (post-A2A) =======================
            at_tiles = {}

            def proj_load(g, engine=None):
                at = atp.tile([P, NCHUNK, P], bf16, tag="at", name=f"at{g}")
                (engine or nc.sync).dma_start(
                    at[:], a2a_out[g][:].rearrange("k p q -> p k q"))
                at_tiles[g] = at

            def proj_mm_items(g):
                gi = GROUPS[g]
                M = 64 * len(gi)
                state = {}
                items = []

                def mk_mm(dc, t0):
                    def run():
                        if t0 == 0:
                            state[dc] = psA.tile([M, CH], f32, tag="qkv",
                                                 name=f"po{g}_{dc}")
                            if has_bp:
                                nc.tensor.matmul(
                                    state[dc][:], ones_r_sb[0:1, 0:M],
                                    bp_sb[0:1, CH * dc:CH * (dc + 1)],
                                    start=True, stop=False)
                        po = state[dc]
                        at = at_tiles[g]
                        dsl = slice(CH * dc, CH * (dc + 1))
                        for t in range(t0, t0 + 4):
                            nc.tensor.matmul(po[:], at[:, t, :],
                                             wp_sb[:, t, dsl],
                                             start=(t == 0 and not has_bp),
                                             stop=(t == NCHUNK - 1))
                    return run

                def mk_evict(dc):
                    def run():
                        po = state.pop(dc)
                        o_sb = outpool.tile([M, CH], f32, tag="out")
                        nc.vector.tensor_copy(o_sb[:], po[:])
                        for i, c in enumerate(gi):
                            nc.sync.dma_start(
                                out_ext.ap()[c, :, CH * dc:CH * (dc + 1)],
                                o_sb[64 * i:64 * i + 64, :])
                    return run

                for dc in range(2):
                    items.append((2048, mk_mm(dc, 0)))
                    items.append((2048, mk_mm(dc, 4)))
                    items.append((0, mk_evict(dc)))
                return items

            def proj_items(g):
                return [(0, lambda g=g: proj_load(g))] + proj_mm_items(g)

            # ================= attention primitives ========================
            def geom(c, kt):
                j = kt - KT_PER_CH * c
                qoff = P * j if j >= 0 else 0
                return qoff, CH - qoff

            def emit_sc(c, kt):
                kc, kb = divmod(kt, KT_PER_CH)
                qoff, n = geom(c, kt)
                sc = psS.tile([P, 2 * CH], f32, tag="sc", name=f"sc{c}_{kt}")
                for h in range(2):
                    lo, hi = 64 * h, 64 * h + 64
                    nc.tensor.matmul(
                        sc[:, CH * h:CH * h + n],
                        qkt_tiles[kc][lo:hi, CH + P * kb:CH + P * (kb + 1)],
                        qkt_tiles[c][lo:hi, qoff:CH],
                        start=True, stop=True,
                    )
                return sc

            def emit_exp(c, kt, sc):
                qoff, n = geom(c, kt)
                pr = probsp.tile([P, 2 * CH], bf16, tag="pr")
                if n == CH:
                    nc.scalar.activation(pr[:, 0:2 * CH], sc[:, 0:2 * CH],
                                         AF.Exp, scale=0.125)
                else:
                    nc.scalar.activation(
                        pr[:].rearrange("p (s q) -> p s q", s=2)[:, :, 0:n],
                        sc[:].rearrange("p (s q) -> p s q", s=2)[:, :, 0:n],
                        AF.Exp, scale=0.125)
                if kt >= KT_PER_CH * c:
                    for h in range(2):
                        nc.vector.tensor_mul(
                            pr[:, CH * h:CH * h + P],
                            pr[:, CH * h:CH * h + P],
                            masks_sb[:])
                return pr

            def emit_av(c, kt, pr, av, nkt):
                kc, kb = divmod(kt, KT_PER_CH)
                qoff, n = geom(c, kt)
                for h in range(2):
                    nc.tensor.matmul(
                        av[h][0:65, qoff:CH],
                        v_tiles[kc][:, kb, 65 * h:65 * h + 65],
                        pr[:, CH * h:CH * h + n],
                        start=(kt == 0), stop=(kt == nkt - 1),
                    )

            # ================= normalize + ship ============================
            def normalize_and_ship(c, av):
                """Full-chunk normalize (chunks 0..6): gpsimd broadcast."""
                g, half = CHUNK_GROUP[c]
                avs = []
                for h in range(2):
                    a = attnp.tile([65, CH], f32, tag="avs")
                    nc.vector.tensor_copy(a[:], av[h][0:65, :])
                    avs.append(a)
                dn = smallp.tile([64, CH], f32, tag="dn")
                for h in range(2):
                    nc.vector.tensor_copy(dn[32 * h:32 * h + 1, :],
                                          avs[h][64:65, :])
                tr = smallp.tile([64, CH], f32, tag="tr")
                nc.vector.transpose(tr[:], dn[:])
                rt = smallp.tile([64, CH], f32, tag="rt")
                nc.vector.reciprocal(
                    rt[:].rearrange("p (k j) -> p k j", j=32)[:, :, 0:1],
                    tr[:].rearrange("p (k j) -> p k j", j=32)[:, :, 0:1])
                rb2 = smallp.tile([64, CH], f32, tag="rb2")
                nc.vector.transpose(rb2[:], rt[:])
                rfix = smallp.tile([1, CH], f32, tag="rfix")
                nc.vector.tensor_copy(rfix[:], rb2[32:33, :])
                for h in range(2):
                    rb = smallp.tile([64, CH], f32, tag="rb")
                    nc.gpsimd.partition_broadcast(
                        rb[:], rfix[:] if h else rb2[0:1, :])
                    attn = attnp.tile([64, CH], bf16, tag="attn")
                    nc.vector.tensor_mul(attn[:], avs[h][0:64, :], rb[:])
                    nc.sync.dma_start(
                        a2a_in[g][:, 64 * h:64 * h + 64,
                                  64 * half:64 * half + 64]
                        .rearrange("k p q -> p k q"),
                        attn[:].rearrange("p (k q) -> p k q", k=N_CORES))

            def tail_wave(av, q0, q1):
                """Chunk-7 ship [q0, q1): av PSUM read directly; head-b's
                denominator staging and the rfix copy run on gpsimd so they
                overlap the DVE chain; per-head ships split across the sync
                and gpsimd queues."""
                g, half = CHUNK_GROUP[NCHUNK - 1]
                w = q1 - q0
                dn = smallp.tile([64, CH], f32, tag="dn")
                for h in range(2):
                    nc.vector.tensor_copy(dn[32 * h:32 * h + 1, 0:w],
                                          av[h][64:65, q0:q1])
                tr = smallp.tile([64, CH], f32, tag="tr")
                nc.vector.transpose(tr[:, 0:w], dn[:, 0:w])
                rt = smallp.tile([64, CH], f32, tag="rt")
                nc.vector.reciprocal(
                    rt[:, 0:w].rearrange("p (k j) -> p k j", j=32)[:, :, 0:1],
                    tr[:, 0:w].rearrange("p (k j) -> p k j", j=32)[:, :, 0:1])
                rb2 = smallp.tile([64, CH], f32, tag="rb2")
                nc.vector.transpose(rb2[:, 0:w], rt[:, 0:w])
                # partition_broadcast reads partition 0 only (verified: a
                # base-partition-32 AP broadcasts garbage), so head-b's
                # recip row must be staged to partition 0 first
                rfix = smallp.tile([1, CH], f32, tag="rfix")
                nc.vector.tensor_copy(rfix[:, 0:w], rb2[32:33, 0:w])
                k0, k1 = q0 // 64, q1 // 64
                for h in range(2):
                    rb = smallp.tile([64, CH], f32, tag="rb")
                    nc.gpsimd.partition_broadcast(
                        rb[:, 0:w], rfix[:, 0:w] if h else rb2[0:1, 0:w])
                    attn = attnp.tile([64, CH], bf16, tag="attn")
                    nc.vector.tensor_mul(attn[:, 0:w], av[h][0:64, q0:q1],
                                         rb[:, 0:w])
                    eng = nc.sync if h == 0 else nc.gpsimd
                    eng.dma_start(
                        a2a_in[g][k0:k1, 64 * h:64 * h + 64,
                                  64 * half:64 * half + 64]
                        .rearrange("k p q -> p k q"),
                        attn[:, 0:w].rearrange("p (k q) -> p k q",
                                               k=k1 - k0))

            def trigger_a2a(g):
                nc.gpsimd.collective_compute(
                    "AllToAll", mybir.AluOpType.bypass,
                    ins=[a2a_in[g][:]], outs=[a2a_out[g][:]],
                    replica_groups=[list(range(N_CORES))],
                )

            # ================= global filler queue =========================
            # Tag-based deadlines: emitting sc(c, 0) before chunk c's Q/K
            # items are popped (or av(c, 0) before its V items) would
            # deadlock the PE stream, so those points force-drain by tag.
            filler = []          # list of (cost_cycles, closure, tag)

            def feed(tag, items):
                filler.extend((cost, fn, tag) for cost, fn in items)

            def drain(budget=None):
                spent = 0
                while filler and (budget is None or spent < budget):
                    cost, fn, _ = filler.pop(0)
                    fn()
                    spent += cost
                return spent

            def drain_tag(tag):
                while any(t == tag for _, _, t in filler):
                    cost, fn, _ = filler.pop(0)
                    fn()

            # ================= main schedule ===============================
            # Startup: interleave wq/wk with the x(0) quarter loads so the
            # first Q/K matmuls start as soon as possible; prestage V(0),
            # QKV(1), QKV(2) as back-to-back PE work under the tiny first
            # exp windows.
            nc.sync.dma_start(wq_sb[:], wq_ext.ap().rearrange("t p c -> p t c"))
            emit_x_load(0, quarters=True)
            nc.sync.dma_start(wk_sb[:], wk_ext.ap().rearrange("t p c -> p t c"))
            nc.sync.dma_start(masks_sb[:], masks_dram.ap())
            nc.sync.dma_start(wv_sb[:], wv_ext.ap().rearrange("t p c -> p t c"))
            # PE warmup: ~3us of back-to-back throwaway matmuls while x(0)
            # streams in, so the first projections run at full clock instead
            # of the cold pstate (the PE ramps only after ~3us gap-free).
            warm_ps = psA.tile([P, P], f32, tag="qkv", name="warm")
            for i in range(24):
                nc.tensor.matmul(warm_ps[:], ones_bf_sb[0:1, :],
                                 ones_bf_sb[0:1, :],
                                 start=(i == 0), stop=(i == 23))
            warm_sb = const.tile([1, 1], f32)
            nc.vector.tensor_copy(warm_sb[:], warm_ps[0:1, 0:1])
            for _, fn in aq_items(0):
                fn()
            # chunk 0's K projection with a split eviction: kt=0's 128 key
            # columns evict first so sc(0, 0) — and with it the first exp —
            # isn't gated on the full 512-column eviction
            psk0 = psA.tile([P, CH], f32, tag="qkv", name="psk0")
            for t in range(NCHUNK):
                nc.tensor.matmul(psk0[:], wk_sb[:, t, :],
                                 x_tiles_all[0][:, t, :],
                                 start=(t == 0), stop=(t == NCHUNK - 1))
            if has_bq:
                nc.scalar.activation(qkt_tiles[0][:, CH:2 * CH], psk0[:],
                                     AF.Copy, bias=bq_sb[:, 1][:, None])
            else:
                nc.vector.tensor_copy(qkt_tiles[0][:, CH:CH + P],
                                      psk0[:, 0:P])
                nc.vector.tensor_copy(qkt_tiles[0][:, CH + P:2 * CH],
                                      psk0[:, P:CH])
            emit_x_load(1)
            sc_next = emit_sc(0, 0)      # first scores -> scalar can start
            for _, fn in v_items(0):     # V(0): needed before av(0, 0)
                fn()
            emit_x_load(2)
            for _, fn in aq_items(1) + k_items(1) + v_items(1):
                fn()
            feed(("qk", 2), aq_items(2) + k_items(2))
            feed(("v", 2), v_items(2))

            def iter_budget(c):
                return 500 if c == 0 else (650 if c == 1 else 800)

            for c in range(NCHUNK):
                nkt = KT_PER_CH * (c + 1)
                av = [psV.tile([P, CH], f32, tag="av", name=f"av{c}_{h}")
                      for h in range(2)]
                if c >= 1 and c + 2 < NCHUNK:
                    emit_x_load(c + 2)
                    feed(("qk", c + 2), aq_items(c + 2) + k_items(c + 2))
                    feed(("v", c + 2), v_items(c + 2))
                if c == 1:
                    # 2MB wp load after x(3) on sync: off the startup rush,
                    # ready well before proj(0) (woven from chunk 4)
                    nc.sync.dma_start(
                        wp_sb[:], wp_ext.ap().rearrange("t p c -> p t c"))
                # Only the a2a_out loads happen mid-kernel (as late as
                # possible: a straggler core makes an A2A finish late, and a
                # proj at-load reached too early blocks the engine queue
                # behind it). ALL proj matmuls run in the tail: the last
                # A2A's completion is skew-dominated (15-20us of otherwise
                # dead PE time in every measured run), so the projections of
                # groups 0-2 overlap that wait and keep the PE ramped for
                # proj(3).
                if c == 5:
                    proj_load(0)
                if c == 7:
                    proj_load(1)
                    proj_load(2)

                prev = None          # (kt, pr) pending av
                sc_cur = sc_next
                for kt in range(nkt):
                    pr = emit_exp(c, kt, sc_cur)
                    if kt + 1 < nkt:
                        sc_cur = emit_sc(c, kt + 1)
                    elif c + 1 < NCHUNK:
                        drain_tag(("qk", c + 1))
                        sc_next = emit_sc(c + 1, 0)
                    if prev is not None:
                        if prev[0] == 0:
                            drain_tag(("v", c))
                        emit_av(c, prev[0], prev[1], av, nkt)
                    prev = (kt, pr)
                    drain(iter_budget(c))
                emit_av(c, prev[0], prev[1], av, nkt)

                if c < NCHUNK - 1:
                    normalize_and_ship(c, av)
                    g, half = CHUNK_GROUP[c]
                    if half == len(GROUPS[g]) - 1:
                        trigger_a2a(g)
                else:
                    # one fast full-width ship: 4 per-diagonal waves all gate
                    # on the final av anyway and quadruple the fixed costs
                    tail_wave(av, 0, CH)
                    trigger_a2a(len(GROUPS) - 1)
                    proj_load(len(GROUPS) - 1, engine=nc.gpsimd)
                    drain(None)
                    for g in range(len(GROUPS)):
                        for _, fn in proj_mm_items(g):
                            fn()

    nc.compile()
    _BUILD_CACHE[key] = nc
    return nc


def _prep_in_maps(x, Wqkv, bqkv, Wproj, bproj):
    x = np.asarray(x, dtype=np.float32)
    Wqkv = np.asarray(Wqkv, dtype=np.float32)
    bqkv = np.asarray(bqkv, dtype=np.float32)
    Wproj = np.asarray(Wproj, dtype=np.float32)
    bproj = np.asarray(bproj, dtype=np.float32)
    xT = np.ascontiguousarray(
        x.reshape(S, D).T.astype(ml_dtypes.bfloat16)
        .reshape(NCHUNK, P, NCHUNK, CH).transpose(2, 0, 1, 3))
    bp = np.ascontiguousarray(bproj.reshape(1, D))
    wp = np.ascontiguousarray(Wproj.astype(ml_dtypes.bfloat16)
                              .reshape(NCHUNK, P, D))
    in_maps = []
    for i in range(N_CORES):
        sl = slice(P * i, P * (i + 1))
        bq = np.stack([bqkv[P * i:P * (i + 1)],
                       bqkv[D + P * i:D + P * (i + 1)],
                       bqkv[2 * D + P * i:2 * D + P * (i + 1)]], axis=1)
        in_maps.append({
            "xT": xT,
            "wq": np.ascontiguousarray(Wqkv[:, sl].astype(ml_dtypes.bfloat16).reshape(NCHUNK, P, P)),
            "wk": np.ascontiguousarray(Wqkv[:, D + P * i:D + P * (i + 1)].astype(ml_dtypes.bfloat16).reshape(NCHUNK, P, P)),
            "wv": np.ascontiguousarray(Wqkv[:, 2 * D + P * i:2 * D + P * (i + 1)].astype(ml_dtypes.bfloat16).reshape(NCHUNK, P, P)),
            "wp": wp,
            "bq": np.ascontiguousarray(bq),
            "bv": bqkv[2 * D + P * i:2 * D + P * (i + 1)].reshape(1, P).astype(ml_dtypes.bfloat16),
            "bp": bp,
        })
    return in_maps


def _run(x, Wqkv, bqkv, Wproj, bproj, trace=False):
    nc = _build(bool(np.any(np.asarray(bqkv))), bool(np.any(np.asarray(bproj))))
    in_maps = _prep_in_maps(x, Wqkv, bqkv, Wproj, bproj)
    res = run_bass_kernel_spmd(nc, in_maps, core_ids=list(range(N_CORES)),
                               trace=trace)
    out = np.empty((NCHUNK, N_CORES, 64, D), dtype=np.float32)
    for c in range(N_CORES):
        out[:, c] = res.results[c]["out"]
    return out.reshape(1, S, D), res


def kernel(x, Wqkv, bqkv, Wproj, bproj):
    out, _ = _run(x, Wqkv, bqkv, Wproj, bproj, trace=False)
    return out
